# revision 12
# baseline (speedup 1.0000x reference)
"""Trainium2 Bass kernel for nn_LocallyConnectedBlock.

Locally-connected conv (5x5, stride 2, SAME) + bias + leaky_relu(0.01) +
BatchNorm (training mode, batch stats over B,OH,OW).

Sharding: spatial over OH, 4 output rows per core x 8 cores, 128 output
positions per core. Compute orientation: out[b, f] per position, with 4
consecutive positions packed onto the 128 PSUM partitions (4 x 32 batch)
via PE column-group tiling. Per quad, the 7 contraction chunks are emitted
t-major (rotating across the 4 column groups) so the 4 quarter-array
matmuls of each chunk run concurrently and LDWEIGHTS prefetch ahead.

Input DMAs are sliced (per-ohl activation slabs, half-group weight tiles)
and issued across both HWDGE queues in compute-need order so the PE starts
~5us in and the stream stays DMA-bound (weights have zero reuse: the
12.6MB/core KM stream is the roofline).

BN batch stats cross 8 cores via a direct SBUF-to-SBUF remote-DMA
all-to-all exchange of per-partition partial sums (XOR-relative dests,
SPMD-uniform), replacing the ncfw mesh AllGather and its ~15-20us ring
latency. Stats finalize runs column-parallel on 64 partitions. y and the
output are bf16 (host upcasts to f32).
"""

import ml_dtypes
import numpy as np

import concourse.bass as bass
import concourse.mybir as mybir
import concourse.tile as tile
from concourse import bacc
from concourse import bass_utils

B, H, W, CIN = 32, 64, 64, 32
KH = KW = 5
F = 64
OH = OW = 32
NCORES = 8
OHL = 4  # output rows per core
NPOS = OHL * OW  # 128 positions per core
NEG_SLOPE = 0.01
EPS = 1e-5
NTOT = float(B * OH * OW)  # BN sample count (32768)
GROUPS = 8
GP = NPOS // GROUPS  # 16 positions per group
QG = GP // 4  # quads per group (4)
NQ = NPOS // 4  # 32 quads per core
NH = 16  # half-group km tiles (8 positions each)
KTP = 33  # tail-chunk contraction: 32 c of tap(4,4) + 1 bias row

F32 = mybir.dt.float32
BF16 = mybir.dt.bfloat16

USE_RDMA = False  # remote-DMA stats exchange; False -> ncfw AllGather


def _marshal(x, kern, bias):
    """Build the 8 per-core input maps (bf16 for matmul operands)."""
    x = np.ascontiguousarray(x, dtype=np.float32)
    kern = np.ascontiguousarray(kern, dtype=np.float32)
    bias = np.ascontiguousarray(bias, dtype=np.float32)

    # SAME padding for 5x5 stride2: pad_lo=1, pad_hi=2 (verified vs jax)
    xp = np.zeros((B, H + 3, W + 3, CIN), np.float32)
    xp[:, 1 : 1 + H, 1 : 1 + W, :] = x
    # patch(oh,ow,kh,kw,c) = xp[:, 2*oh+kh, 2*ow+kw, c]

    kr = kern.reshape(OH, OW, CIN, KH, KW, F)  # c-major fan_in (verified)

    jj = np.arange(4)
    bf = lambda a: np.ascontiguousarray(a.astype(ml_dtypes.bfloat16))
    in_maps = []
    for c in range(NCORES):
        r0 = 8 * c
        # XH[j*32+ci, ohl, w, b] = xp[b, r0+2*ohl+j, w, ci]   (w in 0..66)
        rows = r0 + 2 * jj[None, :] + jj[:, None]  # [j, ohl]
        t = xp[:, rows, 0:67, :]  # [B, j, ohl, 67, CIN]
        xh = np.ascontiguousarray(t.transpose(1, 4, 2, 3, 0)).reshape(128, -1)

        # rows for kh=4 taps
        rw = r0 + 2 * jj + 4  # [ohl]
        t2 = xp[:, rw, :, :]  # [B, ohl, W+3, CIN]
        # XW[j*32+ci, ohl, ow, b] = xp[b, r0+2*ohl+4, 2*ow+j, ci]
        colidx = 2 * np.arange(OW)[None, :] + jj[:, None]  # [j, ow]
        t3 = t2[:, :, colidx, :]  # [B, ohl, j, ow, CIN]
        xw = np.ascontiguousarray(t3.transpose(2, 4, 1, 3, 0)).reshape(128, -1)

        # XR[ci, ohl, ow, b] = xp[b, r0+2*ohl+4, 2*ow+4, ci]; row 32 = 1
        t4 = t2[:, :, 2 * np.arange(OW) + 4, :]  # [B, ohl, ow, CIN]
        xr = np.zeros((KTP, OHL, OW, B), np.float32)
        xr[0:32] = t4.transpose(3, 1, 2, 0)
        xr[32] = 1.0
        xr = xr.reshape(KTP, -1)

        ks = kr[4 * c : 4 * c + 4]  # [ohl, ow, ci, kh, kw, f]
        # KM[j*32+ci, pos, t, f]: t<5 -> (kh=j, kw=t); t=5 -> (kh=4, kw=j)
        km = np.empty((4, 32, OHL, OW, 6, F), np.float32)  # [j, ci, ohl, ow, t, f]
        for tt in range(5):
            km[:, :, :, :, tt, :] = ks[:, :, :, 0:4, tt, :].transpose(3, 2, 0, 1, 4)
        km[:, :, :, :, 5, :] = ks[:, :, :, 4, 0:4, :].transpose(3, 2, 0, 1, 4)
        km = np.ascontiguousarray(km).reshape(128, NPOS, 6, F).reshape(128, -1)

        # KT[p, pos, f]: p<32 tap(4,4); p=32 bias
        kt = np.zeros((KTP, OHL, OW, F), np.float32)
        kt[0:32] = ks[:, :, :, 4, 4, :].transpose(2, 0, 1, 3)
        kt[32] = bias[4 * c : 4 * c + 4]
        kt = kt.reshape(KTP, -1)

        in_maps.append(
            {"XH": bf(xh), "XW": bf(xw), "XR": bf(xr), "KM": bf(km), "KT": bf(kt)}
        )
    return in_maps


def _build_nc():
    nc = bacc.Bacc(
        "TRN2",
        target_bir_lowering=False,
        debug=False,
        enable_asserts=False,
        num_devices=NCORES,
    )
    XH = nc.dram_tensor("XH", [128, OHL * 67 * B], BF16, kind="ExternalInput")
    XW = nc.dram_tensor("XW", [128, OHL * OW * B], BF16, kind="ExternalInput")
    XR = nc.dram_tensor("XR", [KTP, OHL * OW * B], BF16, kind="ExternalInput")
    KM = nc.dram_tensor("KM", [128, NPOS * 6 * F], BF16, kind="ExternalInput")
    KT = nc.dram_tensor("KT", [KTP, NPOS * F], BF16, kind="ExternalInput")
    SCC = nc.dram_tensor("SCC", [F, 1], F32, kind="ExternalInput")
    BBC = nc.dram_tensor("BBC", [F, 1], F32, kind="ExternalInput")
    SCR = nc.dram_tensor("SCR", [1, F], F32, kind="ExternalInput")
    BBR = nc.dram_tensor("BBR", [1, F], F32, kind="ExternalInput")
    EYE = nc.dram_tensor("EYE", [F, F], F32, kind="ExternalInput")
    Y = nc.dram_tensor("Y", [128, NQ * F], BF16, kind="ExternalOutput")

    mult = mybir.AluOpType.mult
    amax = mybir.AluOpType.max
    aadd = mybir.AluOpType.add

    if USE_RDMA:
        rsem = nc.alloc_semaphore("rdma_recv")
        lsem = nc.alloc_semaphore("rdma_local")
        psem = nc.alloc_semaphore("rdma_prep")

    with tile.TileContext(nc) as tc:
        with (
            tc.tile_pool(name="acts", bufs=4) as acts,
            tc.tile_pool(name="singles", bufs=1) as singles,
            tc.tile_pool(name="kmp", bufs=NH) as kmp,
            tc.tile_pool(name="scratch", bufs=2) as scratch,
            tc.tile_pool(name="small", bufs=1) as small,
            tc.tile_pool(name="psum", bufs=4, space=bass.MemorySpace.PSUM) as psp,
            tc.tile_pool(name="pse", bufs=1, space=bass.MemorySpace.PSUM) as pse,
            tc.tile_pool(name="dram", bufs=1, space=bass.MemorySpace.DRAM) as dram,
        ):
            # ---- long-lived comm tiles (stable SBUF; remote writes land
            # in recv while other cores may still be mid-main-loop) ----
            fsq = singles.tile([128, 2 * F], F32, tag="fsq")
            recv = singles.tile([128, NCORES, 2 * F], F32, tag="recv")

            if USE_RDMA:
                # descgen warmup + sem hygiene, then prep the 7 stats sends
                # (descriptors only; data is read at trigger time)
                with tc.tile_critical(name="rdma_prep"):
                    nc.gpsimd.sem_clear(rsem)
                    nc.gpsimd.sem_clear(lsem)
                    nc.gpsimd.sem_clear(psem)
                    warm_prep = nc.gpsimd.remote_sem_update_broadcast(
                        remote_sem=rsem,
                        local_sem=lsem,
                        rdests=[None] * NCORES,
                    )
                    warm_prep.then_inc(psem, 1)
                    nc.gpsimd.wait_ge(psem, 1)
                    nc.gpsimd.trigger_dma(count=1)
                    for k in range(1, NCORES):
                        rd = [None] * NCORES
                        rd[k] = (0, k)
                        prep = nc.gpsimd.remote_dma_broadcast(
                            out_ap=recv[:, k, :],
                            in_ap=fsq[:],
                            remote_sem=rsem,
                            local_sem=lsem,
                            rdests=rd,
                        )
                        prep.then_inc(psem, 1)

            # ---- input DMAs, need-ordered across both HWDGE queues ----
            kmv = KM.ap().rearrange("p (h q t f) -> p h q t f", h=NH, q=GP // 2, t=6)
            xhv = XH.ap().rearrange("p (a b c) -> p a b c", a=OHL, b=67)
            xwv = XW.ap().rearrange("p (a b c) -> p a b c", a=OHL, b=OW)
            xrv = XR.ap().rearrange("p (a b c) -> p a b c", a=OHL, b=OW)

            # sync queue: km half-groups 0..9 (weights stream, the long pole)
            kmh = []
            for h in range(10):
                kt_ = kmp.tile([128, GP // 2, 6, F], BF16, tag="km")
                nc.sync.dma_start(out=kt_[:], in_=kmv[:, h])
                kmh.append(kt_)

            # scalar queue: activation slabs per ohl (first-use order), then
            # km half-groups 10..15 to balance the queues
            xh_t, xw_t, xr_t, ktt = [], [], [], []
            kt = singles.tile([KTP, NPOS, F], BF16, tag="kt")
            ktv = KT.ap().rearrange("p (a b) -> p a b", a=NPOS)
            for ohl in range(OHL):
                xh_ = acts.tile([128, 67, B], BF16, tag="xh")
                nc.scalar.dma_start(out=xh_[:], in_=xhv[:, ohl])
                xh_t.append(xh_)
                xw_ = acts.tile([128, OW, B], BF16, tag="xw")
                nc.scalar.dma_start(out=xw_[:], in_=xwv[:, ohl])
                xw_t.append(xw_)
                xr_ = acts.tile([KTP, OW, B], BF16, tag="xr")
                nc.scalar.dma_start(out=xr_[:], in_=xrv[:, ohl])
                xr_t.append(xr_)
                nc.scalar.dma_start(
                    out=kt[:, 32 * ohl : 32 * ohl + 32, :],
                    in_=ktv[:, 32 * ohl : 32 * ohl + 32],
                )
            for h in range(10, NH):
                kt_ = kmp.tile([128, GP // 2, 6, F], BF16, tag="km")
                nc.scalar.dma_start(out=kt_[:], in_=kmv[:, h])
                kmh.append(kt_)

            sc_col = small.tile([F, 1], F32)
            nc.scalar.dma_start(out=sc_col[:], in_=SCC.ap())
            bb_col = small.tile([F, 1], F32)
            nc.scalar.dma_start(out=bb_col[:], in_=BBC.ap())
            eyesb = small.tile([F, F], F32)
            nc.scalar.dma_start(out=eyesb[:], in_=EYE.ap())

            # warm the ACT Rsqrt table during the prologue (else the BN tail
            # pays the ~1.5us ACT_TABLE_LOAD on the critical path)
            warm = small.tile([1, 1], F32)
            nc.vector.memset(warm[:], 1.0)
            nc.scalar.activation(
                out=warm[:], in_=warm[:],
                func=mybir.ActivationFunctionType.Sqrt,
            )

            # PE warmup: ~20us of dummy matmuls during the prologue DMA so
            # HAM unthrottles the PE clock (1.2 -> 2.4 GHz) before the real
            # matmul stream starts, and stays warm through it.
            wa = small.tile([128, 128], BF16, tag="warm_a")
            nc.vector.memset(wa[:], 0.0)
            wb = small.tile([128, 512], BF16, tag="warm_b")
            nc.vector.memset(wb[:], 0.0)
            wps = pse.tile([128, 512], F32, tag="warm_ps")
            for wi in range(16):
                nc.tensor.matmul(
                    wps[:], wa[:], wb[:], start=(wi == 0), stop=(wi == 15)
                )

            nc.vector.memset(fsq[:], 0.0)

            y_sb = singles.tile([128, NQ, F], BF16, tag="y_sb")
            ones128 = small.tile([128, 1], F32)
            nc.vector.memset(ones128[:], 1.0)
            one1 = small.tile([1, 128], F32)
            nc.vector.memset(one1[:], 1.0)
            epst = small.tile([F, 1], F32)
            nc.vector.memset(epst[:], EPS)

            # ---- main loop: per group, t-major col-rotating matmuls ----
            for g in range(GROUPS):
                ohl = g // 2
                ps = psp.tile([128, QG, F], F32)
                for ql in range(QG):
                    q = g * QG + ql
                    h = q // 2
                    km = kmh[h]
                    for t in range(7):
                        for i in range(4):
                            pos = 4 * q + i
                            pl = pos - 8 * h
                            ow = pos % 32
                            out_sl = ps[32 * i : 32 * i + 32, ql, :]
                            tp = (0, 32 * i)
                            if t < 5:
                                nc.tensor.matmul(
                                    out_sl,
                                    xh_t[ohl][:, 2 * ow + t, :],
                                    km[:, pl, t, :],
                                    start=(t == 0),
                                    stop=False,
                                    tile_position=tp,
                                )
                            elif t == 5:
                                nc.tensor.matmul(
                                    out_sl,
                                    xw_t[ohl][:, ow, :],
                                    km[:, pl, 5, :],
                                    start=False,
                                    stop=False,
                                    tile_position=tp,
                                )
                            else:
                                nc.tensor.matmul(
                                    out_sl,
                                    xr_t[ohl][:, ow, :],
                                    kt[:, pos, :],
                                    start=False,
                                    stop=True,
                                    tile_position=tp,
                                )
                    # full-array dummy matmul per quad keeps the PE HAM
                    # activity monitor at K=8/8 (quarter-array col-masked
                    # MMs alone don't register enough activity)
                    nc.tensor.matmul(
                        wps[:, 0:128], wa[:], wb[:, 0:128], start=True, stop=True
                    )
                # leaky relu drain: y = max(ps, 0.01*ps) -> bf16
                tmp = scratch.tile([128, QG, F], F32, tag="lr")
                nc.scalar.activation(
                    out=tmp[:],
                    in_=ps[:],
                    func=mybir.ActivationFunctionType.Copy,
                    scale=NEG_SLOPE,
                )
                ysl = y_sb[:, g * QG : (g + 1) * QG, :]
                nc.vector.scalar_tensor_tensor(
                    out=ysl, in0=ps[:], scalar=1.0, in1=tmp[:], op0=mult, op1=amax
                )
                # BN partials, accumulated across groups (per partition, per f)
                gsum = scratch.tile([128, F], F32, tag="gsum")
                nc.vector.tensor_reduce(
                    out=gsum[:],
                    in_=ysl.rearrange("p q f -> p f q"),
                    axis=mybir.AxisListType.X,
                    op=aadd,
                )
                nc.vector.tensor_add(fsq[:, 0:F], fsq[:, 0:F], gsum[:])
                sq = scratch.tile([128, QG, F], F32, tag="sq")
                nc.scalar.activation(
                    out=sq[:], in_=ysl, func=mybir.ActivationFunctionType.Square
                )
                gsq = scratch.tile([128, F], F32, tag="gsq")
                nc.vector.tensor_reduce(
                    out=gsq[:],
                    in_=sq[:].rearrange("p q f -> p f q"),
                    axis=mybir.AxisListType.X,
                    op=aadd,
                )
                nc.vector.tensor_add(fsq[:, F : 2 * F], fsq[:, F : 2 * F], gsq[:])

            # ---- BN stats exchange across the 8 cores ----
            tot = small.tile([128, 2 * F], F32, tag="tot")
            if USE_RDMA:
                nc.scalar.activation(
                    out=recv[:, 0, :], in_=fsq[:],
                    func=mybir.ActivationFunctionType.Copy,
                )
                with tc.tile_critical(name="rdma_xchg"):
                    nc.gpsimd.wait_ge(psem, NCORES)
                    nc.gpsimd.trigger_dma(count=NCORES - 1)
                    # 7 senders x (16/8) incs each
                    nc.gpsimd.wait_ge(rsem, 2 * (NCORES - 1))
                    nc.vector.tensor_reduce(
                        out=tot[:],
                        in_=recv[:].rearrange("p s f -> p f s"),
                        axis=mybir.AxisListType.X,
                        op=aadd,
                    )
            else:
                ccin = dram.tile([1, 2 * F], F32)
                ccout = dram.tile([1, NCORES * 2 * F], F32)
                st_ps = pse.tile([1, 2 * F], F32, tag="st_ps")
                nc.tensor.matmul(
                    st_ps[:, 0:F], ones128[:], fsq[:, 0:F], start=True, stop=True
                )
                nc.tensor.matmul(
                    st_ps[:, F : 2 * F], ones128[:], fsq[:, F : 2 * F],
                    start=True, stop=True,
                )
                cc_sb = small.tile([1, 2 * F], F32)
                nc.scalar.activation(
                    out=cc_sb[:], in_=st_ps[:],
                    func=mybir.ActivationFunctionType.Copy,
                )
                nc.sync.dma_start(out=ccin[:], in_=cc_sb[:])
                nc.gpsimd.collective_compute(
                    "AllGather",
                    mybir.AluOpType.bypass,
                    replica_groups=[list(range(NCORES))],
                    ins=[ccin.opt()],
                    outs=[ccout.opt()],
                )
                allst = small.tile([1, NCORES * 2 * F], F32)
                nc.sync.dma_start(out=allst[:], in_=ccout[:])

            # ---- stats finalize ----
            if USE_RDMA:
                # column-parallel on 64 partitions
                pscol = pse.tile([F, 2], F32, tag="pscol")
                nc.tensor.matmul(
                    pscol[:, 0:1], tot[:, 0:F], ones128[:], start=True, stop=True
                )
                nc.tensor.matmul(
                    pscol[:, 1:2], tot[:, F : 2 * F], ones128[:],
                    start=True, stop=True,
                )
                ms = small.tile([F, 2], F32, tag="ms")
                nc.scalar.activation(
                    out=ms[:], in_=pscol[:],
                    func=mybir.ActivationFunctionType.Copy,
                    scale=1.0 / NTOT,
                )
                m2 = small.tile([F, 1], F32, tag="m2")
                nc.vector.tensor_mul(m2[:], ms[:, 0:1], ms[:, 0:1])
                varc = small.tile([F, 1], F32, tag="varc")
                nc.vector.tensor_sub(varc[:], ms[:, 1:2], m2[:])
                abp = small.tile([F, 2], F32, tag="abp")
                sd = small.tile([F, 1], F32, tag="sd")
                nc.scalar.activation(
                    out=sd[:],
                    in_=varc[:],
                    func=mybir.ActivationFunctionType.Sqrt,
                    bias=epst[:],
                    scale=1.0,
                )
                rstd = small.tile([F, 1], F32, tag="rstd")
                nc.vector.reciprocal(out=rstd[:], in_=sd[:])
                nc.vector.tensor_mul(abp[:, 0:1], rstd[:], sc_col[:])  # A
                ma = small.tile([F, 1], F32, tag="ma")
                nc.vector.tensor_mul(ma[:], ms[:, 0:1], abp[:, 0:1])
                nc.vector.tensor_sub(abp[:, 1:2], bb_col[:], ma[:])  # B
                # columns -> rows: [F,1].T @ eye
                psab = pse.tile([1, 2 * F], F32, tag="psab")
                nc.tensor.matmul(
                    psab[:, 0:F], abp[:, 0:1], eyesb[:], start=True, stop=True
                )
                nc.tensor.matmul(
                    psab[:, F : 2 * F], abp[:, 1:2], eyesb[:], start=True, stop=True
                )
                ab2 = small.tile([1, 2 * F], F32, tag="ab2")
                nc.scalar.activation(
                    out=ab2[:], in_=psab[:],
                    func=mybir.ActivationFunctionType.Copy,
                )
            else:
                # row math as in baseline
                totr = small.tile([1, 2 * F], F32, tag="totr")
                nc.vector.tensor_reduce(
                    out=totr[:],
                    in_=allst[:].rearrange("p (r f) -> p f r", r=NCORES),
                    axis=mybir.AxisListType.X,
                    op=aadd,
                )
                ms_r = small.tile([1, 2 * F], F32, tag="msr")
                nc.scalar.activation(
                    out=ms_r[:], in_=totr[:],
                    func=mybir.ActivationFunctionType.Copy,
                    scale=1.0 / NTOT,
                )
                var = small.tile([1, F], F32)
                nc.vector.tensor_mul(var[:], ms_r[:, 0:F], ms_r[:, 0:F])
                nc.vector.tensor_sub(var[:], ms_r[:, F : 2 * F], var[:])
                ep1 = small.tile([1, 1], F32, tag="ep1")
                nc.vector.memset(ep1[:], EPS)
                sdr = small.tile([1, F], F32, tag="sdr")
                nc.scalar.activation(
                    out=sdr[:],
                    in_=var[:],
                    func=mybir.ActivationFunctionType.Sqrt,
                    bias=ep1[:],
                    scale=1.0,
                )
                ab2 = small.tile([1, 2 * F], F32, tag="ab2")
                nc.vector.reciprocal(out=ab2[:, 0:F], in_=sdr[:])
                scr = small.tile([1, F], F32, tag="scr")
                nc.sync.dma_start(out=scr[:], in_=SCR.ap())
                bbr = small.tile([1, F], F32, tag="bbr")
                nc.sync.dma_start(out=bbr[:], in_=BBR.ap())
                nc.vector.tensor_mul(ab2[:, 0:F], scr[:], ab2[:, 0:F])
                nc.vector.tensor_mul(
                    ab2[:, F : 2 * F], ms_r[:, 0:F], ab2[:, 0:F]
                )
                nc.vector.tensor_sub(
                    ab2[:, F : 2 * F], bbr[:], ab2[:, F : 2 * F]
                )

            # broadcast A|B rows to 128 partitions via K=1 matmul, cast bf16
            bc_ps = pse.tile([128, 2 * F], F32, tag="bc_ps")
            nc.tensor.matmul(bc_ps[:], one1[:], ab2[:], start=True, stop=True)
            absb = small.tile([128, 2 * F], BF16, tag="absb")
            nc.scalar.activation(
                out=absb[:], in_=bc_ps[:], func=mybir.ActivationFunctionType.Copy
            )

            # apply in halves: yo = y*A + B, DMA overlaps second half
            a_sl = absb[:, 0:F]
            b_sl = absb[:, F : 2 * F]
            HQ = NQ // 2
            apA = bass.AP(
                tensor=a_sl.tensor,
                offset=a_sl.offset,
                ap=[a_sl.ap[0], [0, HQ], a_sl.ap[1]],
            )
            apB = bass.AP(
                tensor=b_sl.tensor,
                offset=b_sl.offset,
                ap=[b_sl.ap[0], [0, HQ], b_sl.ap[1]],
            )
            yv = Y.ap().rearrange("p (a b) -> p a b", a=NQ)
            for hh in range(2):
                qr = slice(HQ * hh, HQ * (hh + 1))
                yo = scratch.tile([128, HQ, F], BF16, tag="yo")
                nc.vector.scalar_tensor_tensor(
                    out=yo[:], in0=y_sb[:, qr, :], scalar=1.0, in1=apA,
                    op0=mult, op1=mult,
                )
                yo2 = scratch.tile([128, HQ, F], BF16, tag="yo2")
                nc.vector.tensor_add(yo2[:], yo[:], apB)
                nc.sync.dma_start(out=yv[:, qr], in_=yo2[:])

    nc.compile()
    return nc


_NC_CACHE = None
RUN_KWARGS = {}  # test harness may set e.g. {"trace": True}
LAST_RESULT = None


def kernel(x, kernel, bias, scale, bn_bias):
    global _NC_CACHE, LAST_RESULT
    in_maps = _marshal(x, kernel, bias)
    sc = np.ascontiguousarray(np.asarray(scale, np.float32).reshape(F, 1))
    bb = np.ascontiguousarray(np.asarray(bn_bias, np.float32).reshape(F, 1))
    eye = np.ascontiguousarray(np.eye(F, dtype=np.float32))
    for m in in_maps:
        m["SCC"] = sc
        m["BBC"] = bb
        m["SCR"] = np.ascontiguousarray(sc.reshape(1, F))
        m["BBR"] = np.ascontiguousarray(bb.reshape(1, F))
        m["EYE"] = eye

    if _NC_CACHE is None:
        _NC_CACHE = _build_nc()
    nc = _NC_CACHE

    res = bass_utils.run_bass_kernel_spmd(
        nc, in_maps, core_ids=list(range(NCORES)), **RUN_KWARGS
    )
    LAST_RESULT = res

    out = np.empty((B, OH, OW, F), np.float32)
    for c in range(NCORES):
        yc = res.results[c]["Y"].astype(np.float32).reshape(4, B, NQ, F)
        yb = np.transpose(yc, (1, 2, 0, 3)).reshape(B, OHL, OW, F)
        out[:, 4 * c : 4 * c + 4, :, :] = yb
    return out
